# revision 1
# baseline (speedup 1.0000x reference)
"""DCGRU cell Trainium2 kernel.

Math (per batch i):
  xs = [input, state]                                  [N, 66]
  aggr[j] = S[j] @ xs          (J=4 supports)          [N, 66]
  r = sigmoid(sum_j aggr[j] @ Wr[j] + br)              [N, 64]
  u = sigmoid(sum_j aggr[j] @ Wu[j] + bu)
  xc = [input, r*state]
  c = tanh(sum_j (S[j] @ xc) @ Wc[j] + bc)
  out = u*state + (1-u)*c

Sharding: data-parallel over batch, 8 batches per core on 8 cores.
supports/weights replicated. No collectives.

Device kernel layout (per core, Bc=8):
  - Host pre-transposes supports: ST[j] = S[j].T ([m, k], m = contraction),
    cast fp16 -> stationary matmul operands are contiguous row-block slices.
  - XS packed [m=2048, (i=8, f=66)] fp16: moving operand, SBUF resident.
  - Big matmul accumulates aggr[j] = [k, (i,f)] in PSUM over 16 m-blocks;
    528-col batches split 264+264 across two banks; kb/h-major mb-minor
    order so each bank's drain overlaps the next bank's matmuls.
  - PE-transpose [128, 66] slices -> aggT[i][j] [66, 256], W-projection
    (contract 66, fp32r, accumulate over j in PSUM), bias+activation on
    ScalarE -> ru.T [128 = (r|u), k] per batch.
  - Phase 2 identical with xc; c.T overwrites the dead r.T rows.
  - GRU combine on DVE in [64, N] layout; host undoes the final transpose.
"""

import sys

if '/opt/trn_rl_repo' not in sys.path:
    sys.path.insert(0, '/opt/trn_rl_repo')

import numpy as np

B, N, IN, OUT, J = 64, 2048, 2, 64, 4
NCORES = 8
BC = B // NCORES            # 8 batches per core
F = IN + OUT                # 66
CB = BC * F                 # 528 moving columns
P = 128
HALF = CB // 2              # 264 (psum bank split)
NMB = N // P                # 16 m blocks
NKB = N // P                # 16 k blocks
KBG = 2                     # k blocks per psum group
NG = NKB // KBG             # 8 groups
MBQ = 8                     # m blocks per ST dma

MM16 = True                 # fp16 big-matmul operands (vs float32r)

_CACHE = {}


def _build_module():
    import concourse.tile as tile
    import concourse.mybir as mybir
    from concourse import bacc
    from concourse.masks import make_identity

    f32 = mybir.dt.float32
    f32r = mybir.dt.float32r
    mmdt = mybir.dt.float16 if MM16 else f32r
    AF = mybir.ActivationFunctionType

    nc = bacc.Bacc("TRN2", target_bir_lowering=False, debug=False,
                   num_devices=1)

    st_d = nc.dram_tensor("st", [J, N, N], mmdt, kind="ExternalInput").ap()
    xs_d = nc.dram_tensor("xs", [N, CB], mmdt, kind="ExternalInput").ap()
    xin_d = nc.dram_tensor("xin", [N, BC * IN], f32, kind="ExternalInput").ap()
    stT_d = nc.dram_tensor("stT", [BC, OUT, N], f32, kind="ExternalInput").ap()
    wru_d = nc.dram_tensor("wru", [J, F, 2 * OUT], mmdt, kind="ExternalInput").ap()
    wc_d = nc.dram_tensor("wc", [J, F, OUT], mmdt, kind="ExternalInput").ap()
    bru_d = nc.dram_tensor("bru", [2 * OUT, 1], f32, kind="ExternalInput").ap()
    bc_d = nc.dram_tensor("bc", [OUT, 1], f32, kind="ExternalInput").ap()
    outT_d = nc.dram_tensor("outT", [BC, OUT, N], f32, kind="ExternalOutput").ap()

    with tile.TileContext(nc) as tc:
        with tc.tile_pool(name="const", bufs=1) as const_pool, \
             tc.tile_pool(name="xs", bufs=18) as xs_pool, \
             tc.tile_pool(name="xin", bufs=16) as xin_pool, \
             tc.tile_pool(name="ruT", bufs=BC) as ruT_pool, \
             tc.tile_pool(name="stT", bufs=2) as stT_pool:

            ident = const_pool.tile([P, P], mmdt, tag="ident")
            make_identity(nc, ident[:])

            wru_t = []
            wc_t = []
            for j in range(J):
                w1 = const_pool.tile([F, 2 * OUT], mmdt, tag=f"wru{j}")
                nc.sync.dma_start(w1[:], wru_d[j])
                wru_t.append(w1)
                w2 = const_pool.tile([F, OUT], mmdt, tag=f"wc{j}")
                nc.sync.dma_start(w2[:], wc_d[j])
                wc_t.append(w2)
            bru_t = const_pool.tile([2 * OUT, 1], f32, tag="bru")
            nc.sync.dma_start(bru_t[:], bru_d[:])
            bc_t = const_pool.tile([OUT, 1], f32, tag="bc")
            nc.sync.dma_start(bc_t[:], bc_d[:])

            xs_tiles = []
            for mb in range(NMB):
                t = xs_pool.tile([P, CB], mmdt, tag="xs")
                nc.sync.dma_start(t[:], xs_d[mb * P:(mb + 1) * P, :])
                xs_tiles.append(t)
            xin_tiles = []
            for mb in range(NMB):
                t = xin_pool.tile([P, BC * IN], f32, tag="xin")
                nc.sync.dma_start(t[:], xin_d[mb * P:(mb + 1) * P, :])
                xin_tiles.append(t)

            ruT_tiles = [ruT_pool.tile([P, N], f32, tag="ruT", name=f"ruT{i}")
                         for i in range(BC)]

            def big_phase(x_tiles, w_tiles, out_rows, bias_t, act_fn,
                          out_slice_fn):
                """One graph-conv pass + projection + activation.

                out_slice_fn(i, k0, width) -> SBUF AP [out_rows, width]
                receiving act(proj + bias) for batch i, k cols [k0, k0+w).
                """
                with tc.tile_pool(name="stst", bufs=3) as st_pool, \
                     tc.tile_pool(name="agg", bufs=16) as agg_pool, \
                     tc.tile_pool(name="aggT", bufs=8) as aggT_pool, \
                     tc.tile_pool(name="aggps", bufs=4, space="PSUM") as agg_ps_pool, \
                     tc.tile_pool(name="tpps", bufs=4, space="PSUM") as tp_ps_pool:
                    for g in range(NG):
                        k0 = g * KBG * P        # 256-aligned k offset
                        agg_sb = {}
                        for j in range(J):
                            st_ts = []
                            for mq in range(NMB // MBQ):
                                st_t = st_pool.tile([P, MBQ, KBG * P], mmdt,
                                                    tag="st")
                                src = st_d[j, mq * MBQ * P:(mq + 1) * MBQ * P,
                                           k0:k0 + KBG * P]
                                src = src.rearrange("(g p) k -> p g k", p=P)
                                nc.sync.dma_start(st_t[:], src)
                                st_ts.append(st_t)
                            # kb/h-major, mb-minor: each psum tile's
                            # accumulation closes early so its drain overlaps
                            # the next tile's matmuls.
                            for kb in range(KBG):
                                t = agg_pool.tile([P, CB], mmdt, tag="agg",
                                                  name=f"agg{j}_{kb}")
                                for h in range(2):
                                    pst = agg_ps_pool.tile(
                                        [P, HALF], f32, tag="aggps",
                                        name=f"aggps{kb}_{h}")
                                    for mb in range(NMB):
                                        mq, ml = divmod(mb, MBQ)
                                        lhsT = st_ts[mq][:, ml,
                                                         kb * P:(kb + 1) * P]
                                        nc.tensor.matmul(
                                            pst[:],
                                            lhsT,
                                            x_tiles[mb][:, h * HALF:(h + 1) * HALF],
                                            start=(mb == 0),
                                            stop=(mb == NMB - 1),
                                        )
                                    if (kb + h) % 2 == 0:
                                        nc.vector.tensor_copy(
                                            t[:, h * HALF:(h + 1) * HALF],
                                            pst[:])
                                    else:
                                        nc.scalar.copy(
                                            t[:, h * HALF:(h + 1) * HALF],
                                            pst[:])
                                agg_sb[(j, kb)] = t

                        for i in range(BC):
                            aggT = []
                            for j in range(J):
                                tp = tp_ps_pool.tile([F, KBG * P], mmdt,
                                                     tag="tpproj",
                                                     name=f"tp{i}_{j}")
                                for kb in range(KBG):
                                    nc.tensor.transpose(
                                        tp[:, kb * P:(kb + 1) * P],
                                        agg_sb[(j, kb)][:, i * F:(i + 1) * F],
                                        ident[:])
                                at = aggT_pool.tile([F, KBG * P], mmdt,
                                                    tag="aggT",
                                                    name=f"aggT{i}_{j}")
                                if (i + j) % 2 == 0:
                                    nc.vector.tensor_copy(at[:], tp[:])
                                else:
                                    nc.scalar.copy(at[:], tp[:])
                                aggT.append(at)
                            pp = tp_ps_pool.tile([out_rows, KBG * P], f32,
                                                 tag="tpproj",
                                                 name=f"proj{i}")
                            for j in range(J):
                                nc.tensor.matmul(
                                    pp[:],
                                    w_tiles[j][:],
                                    aggT[j][:],
                                    start=(j == 0),
                                    stop=(j == J - 1),
                                )
                            nc.scalar.activation(
                                out_slice_fn(i, k0, KBG * P), pp[:], act_fn,
                                bias=bias_t[:, 0:1])

            # ---- phase 1: r|u = sigmoid(graph_conv(xs, Wr|Wu)) ----
            big_phase(
                xs_tiles, wru_t, 2 * OUT, bru_t, AF.Sigmoid,
                lambda i, k0, w: ruT_tiles[i][:, k0:k0 + w])

            # ---- boundary: xc = [input, r*state] in [m, (i,f)] layout ----
            xc_tiles = [xs_pool.tile([P, CB], mmdt, tag="xs", name=f"xc{mb}")
                        for mb in range(NMB)]
            with tc.tile_pool(name="rstp", bufs=2, space="PSUM") as rstp_pool, \
                 tc.tile_pool(name="rsT", bufs=2) as rsT_pool:
                for mb in range(NMB):
                    dst = xc_tiles[mb][:].rearrange("p (i f) -> p i f", f=F)
                    src = xin_tiles[mb][:].rearrange("p (i f) -> p i f", f=IN)
                    nc.vector.tensor_copy(dst[:, :, 0:IN], src)
                for i in range(BC):
                    stt = stT_pool.tile([OUT, N], f32, tag="stT")
                    nc.sync.dma_start(stt[:], stT_d[i])
                    rst = rsT_pool.tile([OUT, N], mmdt, tag="rsT")
                    nc.vector.tensor_mul(rst[:], ruT_tiles[i][0:OUT, :],
                                         stt[:])
                    for mb in range(NMB):
                        tp = rstp_pool.tile([P, OUT], mmdt, tag="rstp")
                        nc.tensor.transpose(tp[:], rst[:, mb * P:(mb + 1) * P],
                                            ident[0:OUT, 0:OUT])
                        if mb % 2 == 0:
                            nc.vector.tensor_copy(
                                xc_tiles[mb][:, i * F + IN:(i + 1) * F], tp[:])
                        else:
                            nc.scalar.copy(
                                xc_tiles[mb][:, i * F + IN:(i + 1) * F], tp[:])

            # ---- phase 2: c.T = tanh(proj) overwrites dead r.T rows ----
            big_phase(
                xc_tiles, wc_t, OUT, bc_t, AF.Tanh,
                lambda i, k0, w: ruT_tiles[i][0:OUT, k0:k0 + w])

            # ---- GRU combine: out = c + u*(state - c) ----
            with tc.tile_pool(name="tmp", bufs=3) as tmp_pool:
                for i in range(BC):
                    stt = stT_pool.tile([OUT, N], f32, tag="stT")
                    nc.sync.dma_start(stt[:], stT_d[i])
                    u0 = tmp_pool.tile([OUT, N], f32, tag="tmp")
                    # partition-base shift (64 -> 0) needs a DMA, not DVE
                    nc.sync.dma_start(u0[:], ruT_tiles[i][OUT:2 * OUT, :])
                    t1 = tmp_pool.tile([OUT, N], f32, tag="tmp")
                    nc.vector.tensor_sub(t1[:], stt[:], ruT_tiles[i][0:OUT, :])
                    t2 = tmp_pool.tile([OUT, N], f32, tag="tmp")
                    nc.vector.tensor_mul(t2[:], u0[:], t1[:])
                    t3 = tmp_pool.tile([OUT, N], f32, tag="tmp")
                    nc.vector.tensor_add(t3[:], ruT_tiles[i][0:OUT, :], t2[:])
                    nc.sync.dma_start(outT_d[i], t3[:])

    nc.compile()
    return nc


def _get_module():
    if "nc" not in _CACHE:
        _CACHE["nc"] = _build_module()
    return _CACHE["nc"]


def kernel(input, state, supports, Wr, br, Wu, bu, Wc, bc):
    input = np.asarray(input, np.float32)
    state = np.asarray(state, np.float32)
    supports = np.asarray(supports, np.float32)
    Wr = np.asarray(Wr, np.float32)
    br = np.asarray(br, np.float32)
    Wu = np.asarray(Wu, np.float32)
    bu = np.asarray(bu, np.float32)
    Wc = np.asarray(Wc, np.float32)
    bc = np.asarray(bc, np.float32)

    from concourse.bass_utils import run_bass_kernel_spmd

    nc = _get_module()

    mmnp = np.float16 if MM16 else np.float32
    st_host = np.ascontiguousarray(supports.transpose(0, 2, 1).astype(mmnp))
    wru = np.ascontiguousarray(np.concatenate([Wr, Wu], axis=2).astype(mmnp))
    bru = np.concatenate([br, bu]).reshape(2 * OUT, 1).astype(np.float32)
    bcc = bc.reshape(OUT, 1).astype(np.float32)
    xs_full = np.concatenate([input, state], axis=2)  # [B, N, F]

    in_maps = []
    for c in range(NCORES):
        sl = slice(c * BC, (c + 1) * BC)
        xs_c = np.ascontiguousarray(
            xs_full[sl].transpose(1, 0, 2).reshape(N, CB).astype(mmnp))
        xin_c = np.ascontiguousarray(
            input[sl].transpose(1, 0, 2).reshape(N, BC * IN))
        stT_c = np.ascontiguousarray(state[sl].transpose(0, 2, 1))
        in_maps.append({
            "st": st_host,
            "xs": xs_c,
            "xin": xin_c,
            "stT": stT_c,
            "wru": wru,
            "wc": np.ascontiguousarray(Wc.astype(mmnp)),
            "bru": bru,
            "bc": bcc,
        })

    import time
    t0 = time.monotonic()
    res = run_bass_kernel_spmd(nc, in_maps, core_ids=list(range(NCORES)))
    _CACHE["last_wall_s"] = time.monotonic() - t0

    out = np.empty((B, N, OUT), np.float32)
    for c in range(NCORES):
        outT = res.results[c]["outT"]           # [BC, OUT, N]
        out[c * BC:(c + 1) * BC] = outT.transpose(0, 2, 1)
    return out



# revision 43
# speedup vs baseline: 1.4845x; 1.4845x over previous
"""DCGRU cell Trainium2 kernel (fp8 DoubleRow aggregation).

Math (per batch i):
  xs = [input, state]                                  [N, 66]
  aggr[j] = S[j] @ xs          (J=4 supports)          [N, 66]
  r = sigmoid(sum_j aggr[j] @ Wr[j] + br)              [N, 64]
  um = 1 - sigmoid(sum_j aggr[j] @ Wu[j] + bu)         (= 1-u, via scale=-1)
  xc = [input, r*state]
  c = tanh(sum_j (S[j] @ xc) @ Wc[j] + bc)
  out = (1-um)*state + um*c = state + um*(c - state)

Sharding: data-parallel over batch, 8 batches per core on 8 cores.
supports/weights replicated. No collectives.

Device kernel (per core, Bc=8):
  - Supports prescaled x64, cast fp8e4 (e4m3), host-swizzled to
    [j, g, p, mq, pair, k] so each [128, mq, 2, 256] stationary tile DMA
    is fully contiguous per partition (4KB lines).
  - xs packed [p, mq, pair, (i,f)] fp8 in ONE tile / one DMA.
  - Aggregation: DoubleRow fp8 matmuls contract two 128-m tiles per
    instruction at 0.5 cyc/col; psum [128, 264] halves accumulate over
    8 pair-instructions; drains to fp16 (x64 scale folded into W/64).
  - Tail per group g (pipelined one group behind agg): per batch i issue
    act(i-2) -> transposes(i) + drains -> proj(i-1), so PE never waits on
    the cross-engine drain/act latency.
  - post_act hook spreads phase-boundary and combine work across groups:
    phase 1: (r*state).T chunk -> transpose -> xc fp8 cols;
    phase 2: out chunk = state + um*(c - state), DMA at last group.
"""

import sys

if '/opt/trn_rl_repo' not in sys.path:
    sys.path.insert(0, '/opt/trn_rl_repo')

import numpy as np
import ml_dtypes

B, N, IN, OUT, J = 64, 2048, 2, 64, 4
NCORES = 8
BC = B // NCORES            # 8 batches per core
F = IN + OUT                # 66
CB = BC * F                 # 528 moving columns
P = 128
HALF = CB // 2              # 264 (psum bank split)
NG = 8                      # k groups (256 cols each)
KBG = 2                     # k blocks per group
GW = KBG * P                # 256 cols per group
MQ = 8                      # m pair-tiles (each pair = 256 rows)
SCALE = 64.0                # fp8 support prescale (folded out via W/SCALE)

_CACHE = {}


def _build_module():
    import concourse.tile as tile
    import concourse.mybir as mybir
    from concourse import bacc
    from concourse.masks import make_identity

    f32 = mybir.dt.float32
    f16 = mybir.dt.float16
    f8 = mybir.dt.float8e4
    AF = mybir.ActivationFunctionType
    PM = mybir.MatmulPerfMode.DoubleRow

    nc = bacc.Bacc("TRN2", target_bir_lowering=False, debug=False,
                   num_devices=1)

    st_d = nc.dram_tensor("st", [J, NG, P, 2 * MQ, 3, GW], f8,
                          kind="ExternalInput").ap()
    xs_d = nc.dram_tensor("xs", [P, 2, 2 * MQ, 2, HALF], f8,
                          kind="ExternalInput").ap()
    stT_d = nc.dram_tensor("stT", [BC, OUT, N], f16, kind="ExternalInput").ap()
    wcat_d = nc.dram_tensor("wcat", [F, J * 2 * OUT + J * OUT], f16,
                            kind="ExternalInput").ap()
    bvec_d = nc.dram_tensor("bvec", [P, 3], f32, kind="ExternalInput").ap()
    outT_d = nc.dram_tensor("outT", [BC, OUT, N], f16,
                            kind="ExternalOutput").ap()

    with tile.TileContext(nc) as tc:
        with tc.tile_pool(name="const", bufs=1) as const_pool, \
             tc.tile_pool(name="xs", bufs=1) as xs_pool, \
             tc.tile_pool(name="xc", bufs=1) as xc_pool, \
             tc.tile_pool(name="ruT", bufs=BC) as ruT_pool, \
             tc.tile_pool(name="stT", bufs=BC) as stT_pool, \
             tc.tile_pool(name="um0", bufs=BC) as um0_pool, \
             tc.tile_pool(name="rsT", bufs=2) as rsT_pool, \
             tc.tile_pool(name="cscr", bufs=2) as cscr_pool:

            # xs first (biggest serial dependency for the first matmuls);
            # wcat/bvec DMAs deferred into phase1_extra(0) behind the g0
            # support tiles.
            xs_t = xs_pool.tile([P, 2, 2 * MQ, 2, HALF], f8, tag="xs")
            nc.sync.dma_start(xs_t[:, 0], xs_d[:, 0])
            nc.sync.dma_start(xs_t[:, 1], xs_d[:, 1])

            def xs_load():
                pass
            wcat_t = const_pool.tile([F, J * 2 * OUT + J * OUT], f16,
                                     tag="wcat")
            bvec_t = const_pool.tile([P, 3], f32, tag="bvec")

            ident = const_pool.tile([P, P], f16, tag="ident")
            make_identity(nc, ident[:])

            wru_t = [wcat_t[:, j * 2 * OUT:(j + 1) * 2 * OUT]
                     for j in range(J)]
            wc_t = [wcat_t[:, J * 2 * OUT + j * OUT:J * 2 * OUT + (j + 1) * OUT]
                    for j in range(J)]
            bru_ap = bvec_t[:, 0:1]
            sgn_ap = bvec_t[:, 1:2]
            bc_ap = bvec_t[0:OUT, 2:3]

            xc_t = xc_pool.tile([P, 2, 2 * MQ, 2, HALF], f8, tag="xc")
            for h in range(2):
                dst = xc_t[:, h].rearrange("p m two (i f) -> p m two i f", f=F)
                src = xs_t[:, h].rearrange("p m two (i f) -> p m two i f", f=F)
                nc.gpsimd.tensor_copy(dst[:, :, :, :, 0:IN],
                                      src[:, :, :, :, 0:IN])

            stT_tiles = [stT_pool.tile([OUT, N], f16, tag="stT",
                                       name=f"stT{i}")
                         for i in range(BC)]
            ruT_tiles = [ruT_pool.tile([P, N], f16, tag="ruT", name=f"ruT{i}")
                         for i in range(BC)]
            um0_tiles = [um0_pool.tile([OUT, N], f16, tag="um0",
                                       name=f"um0{i}")
                         for i in range(BC)]

            def big_phase(x_t, w_tiles, out_rows, bias_ap, act_fn,
                          scale_ap, out_slice_fn, agg_rr, aggT_rr,
                          extra_fn=None, post_act=None, tp_bufs=2,
                          pre_fn=None):
                """One graph-conv pass + projection + activation.

                Pipelined: agg(g) is issued before tail(g-1); within a tail,
                act(i-2) -> post_act(i-2) -> transposes(i)+drains ->
                proj(i-1). Queue discipline: gp takes the slack-tolerant agg
                psum drains; scalar/DVE take the latency-critical tail work;
                post_act chains stay within one queue so they never block a
                drain at an in-order queue head.
                """
                with tc.tile_pool(name="stst", bufs=4) as st_pool, \
                     tc.tile_pool(name="agg", bufs=16) as agg_pool, \
                     tc.tile_pool(name="aggT", bufs=4) as aggT_pool, \
                     tc.tile_pool(name="aggps", bufs=3, space="PSUM") as agg_ps_pool, \
                     tc.tile_pool(name="tpps", bufs=tp_bufs, space="PSUM") as tp_ps_pool, \
                     tc.tile_pool(name="pjps", bufs=2, space="PSUM") as pj_ps_pool:

                    nd = 0
                    td = 0
                    pend_proj = [None]
                    pend_act = [None]

                    def agg_block(g, j, pre_fn=None):
                        # 3-product compensated fp8 aggregation for one
                        # support: psum = Sh16@xh (m-pairs, slot0) +
                        # (Sh@xl + Sl@xh) per m-tile (slots 1,2 x 0,1);
                        # drain scales by 1/16 -> aggr = 64*S@x in f16.
                        nonlocal nd
                        blk = {}
                        st_t = st_pool.tile([P, 2 * MQ, 3, GW], f8, tag="st")
                        nc.sync.dma_start(st_t[:], st_d[j, g])
                        if pre_fn is not None:
                            pre_fn()
                        for kb in range(KBG):
                            t = agg_pool.tile([P, CB], f16, tag="agg",
                                              name=f"agg{j}_{kb}")
                            for h in range(2):
                                pst = agg_ps_pool.tile(
                                    [P, HALF], f32, tag="aggps",
                                    name=f"aggps{kb}_{h}")
                                for mq in range(MQ):
                                    nc.tensor.matmul(
                                        pst[:],
                                        st_t[:, 2 * mq:2 * mq + 2, 0,
                                             kb * P:(kb + 1) * P],
                                        x_t[:, h, 2 * mq:2 * mq + 2, 1, :],
                                        start=(mq == 0),
                                        stop=False,
                                        perf_mode=PM,
                                    )
                                for mt in range(2 * MQ):
                                    nc.tensor.matmul(
                                        pst[:],
                                        st_t[:, mt, 1:3,
                                             kb * P:(kb + 1) * P],
                                        x_t[:, h, mt, :, :],
                                        start=False,
                                        stop=(mt == 2 * MQ - 1),
                                        perf_mode=PM,
                                    )
                                eng = agg_rr[nd % len(agg_rr)]
                                nd += 1
                                eng(t[:, h * HALF:(h + 1) * HALF], pst[:])
                            blk[(j, kb)] = t
                        return blk

                    def one_act(g):
                        pi, ppp = pend_act[0]
                        nc.scalar.activation(
                            out_slice_fn(pi, g * GW, GW), ppp[:],
                            act_fn, bias=bias_ap, scale=scale_ap)
                        pend_act[0] = None
                        if post_act is not None:
                            post_act(pi, g, tp_ps_pool)

                    def issue_proj(g):
                        pi, tpair = pend_proj[0]
                        pp = pj_ps_pool.tile([out_rows, GW], f32, tag="pj",
                                             name=f"proj{pi}")
                        for j in range(J):
                            nc.tensor.matmul(
                                pp[:], w_tiles[j][:, 0:out_rows],
                                tpair[j // 2][:, j % 2],
                                start=(j == 0), stop=(j == J - 1))
                        pend_proj[0] = None
                        pend_act[0] = (pi, pp)

                    def tail_iter(g, i, agg_sb):
                        nonlocal td
                        if pend_act[0] is not None:
                            one_act(g)      # act(i-2)
                        tpair = []
                        for half in range(2):
                            tp = tp_ps_pool.tile([F, 2, GW], f16,
                                                 tag="tppair",
                                                 name=f"tp{i}_{half}")
                            for jj in range(2):
                                j = half * 2 + jj
                                for kb in range(KBG):
                                    nc.tensor.transpose(
                                        tp[:, jj, kb * P:(kb + 1) * P],
                                        agg_sb[(j, kb)][:, i * F:(i + 1) * F],
                                        ident[:])
                            at = aggT_pool.tile([F, 2, GW], f16, tag="aggT",
                                                name=f"aggT{i}_{half}")
                            aggT_rr[td % len(aggT_rr)](at[:], tp[:])
                            td += 1
                            tpair.append(at)
                        if pend_proj[0] is not None:
                            issue_proj(g)   # proj(i-1)
                        pend_proj[0] = (i, tpair)

                    def tail_flush(g):
                        # act(BC-2), proj(BC-1), act(BC-1)
                        if pend_act[0] is not None:
                            one_act(g)
                        issue_proj(g)
                        one_act(g)

                    # Interleaved issue: each engine queue sees work in
                    # (approximate) temporal execution order, so no queue
                    # head ever waits on far-future work while near-term
                    # work sits behind it.
                    agg_prev = None
                    for g in range(NG):
                        agg_cur = {}
                        for j in range(J):
                            agg_cur.update(agg_block(
                                g, j,
                                pre_fn if (g == 0 and j == 0) else None))
                            if j == 0 and extra_fn is not None:
                                extra_fn(g)
                            if agg_prev is not None:
                                tail_iter(g - 1, 2 * j, agg_prev)
                                tail_iter(g - 1, 2 * j + 1, agg_prev)
                        if agg_prev is not None:
                            tail_flush(g - 1)
                        agg_prev = agg_cur
                    for i in range(BC):
                        tail_iter(NG - 1, i, agg_prev)
                    tail_flush(NG - 1)

            # ---------------- phase 1 ----------------
            def phase1_extra(g):
                if g == 0:
                    nc.sync.dma_start(wcat_t[:], wcat_d[:])
                    nc.sync.dma_start(bvec_t[:], bvec_d[:])
                if g <= 1:
                    for i in range(g * 4, g * 4 + 4):
                        nc.sync.dma_start(stT_tiles[i][:], stT_d[i])

            rst_n = [0]

            def phase1_post_act(i, g, tp_pool):
                # (r*state).T chunk, split hi/lo, transposed into xc fp8:
                #   rsh8 = fp8(rst); res = rst - f16(rsh8); xl = fp8(16*res)
                # hi and res transposed separately; drains convert to fp8.
                k0 = g * GW
                h, il = divmod(i, 4)
                rst = rsT_pool.tile([OUT, GW], f16, tag="rsT")
                nc.vector.tensor_mul(rst[:], ruT_tiles[i][0:OUT, k0:k0 + GW],
                                     stT_tiles[i][:, k0:k0 + GW])
                rsh8 = rsT_pool.tile([OUT, GW], f8, tag="rs8")
                nc.vector.tensor_copy(rsh8[:], rst[:])
                rsh16 = rsT_pool.tile([OUT, GW], f16, tag="rsH")
                nc.vector.tensor_copy(rsh16[:], rsh8[:])
                res = rsT_pool.tile([OUT, GW], f16, tag="rsR")
                nc.vector.tensor_sub(res[:], rst[:], rsh16[:])
                tph = tp_pool.tile([P, KBG, OUT], f16, tag="tppair",
                                   name=f"rsth{i}")
                tpl = tp_pool.tile([P, KBG, OUT], f16, tag="tppair",
                                   name=f"rstl{i}")
                for kb in range(KBG):
                    nc.tensor.transpose(tph[:, kb],
                                        rsh16[:, kb * P:(kb + 1) * P],
                                        ident[0:OUT, 0:OUT])
                    nc.tensor.transpose(tpl[:, kb],
                                        res[:, kb * P:(kb + 1) * P],
                                        ident[0:OUT, 0:OUT])
                for kb in range(KBG):
                    mt = g * KBG + kb
                    ds = xc_t[:, h, mt, :, il * F + IN:(il + 1) * F]
                    if rst_n[0] % 2 == 0:
                        nc.vector.tensor_copy(ds[:, 1], tph[:, kb])
                        nc.scalar.activation(ds[:, 0], tpl[:, kb], AF.Copy,
                                             scale=16.0)
                    else:
                        nc.scalar.copy(ds[:, 1], tph[:, kb])
                        nc.vector.tensor_scalar_mul(ds[:, 0], tpl[:, kb],
                                                    16.0)
                    rst_n[0] += 1

            def v_drain(dst, src_ap):
                nc.vector.tensor_scalar_mul(dst, src_ap, 1.0 / 16.0)

            def s_drain(dst, src_ap):
                nc.scalar.activation(dst, src_ap, AF.Copy, scale=1.0 / 16.0)

            big_phase(
                xs_t, wru_t, 2 * OUT, bru_ap, AF.Sigmoid, sgn_ap,
                lambda i, k0, w: ruT_tiles[i][:, k0:k0 + w],
                agg_rr=[v_drain, s_drain],
                aggT_rr=[nc.vector.tensor_copy, nc.scalar.copy],
                extra_fn=phase1_extra,
                post_act=phase1_post_act,
                tp_bufs=3,
                pre_fn=xs_load)

            # um = (1-u).T needs partition base 0: SBUF->SBUF DMA via the
            # Activation queue (SP is busy prefetching phase-2 supports).
            for i in range(BC):
                nc.scalar.dma_start(um0_tiles[i][:],
                                    ruT_tiles[i][OUT:2 * OUT, :])

            # ---------------- phase 2 ----------------
            def phase2_post_act(i, g, pj_pool):
                # out chunk = state + um*(c - state), in place over stT.
                # Whole chain on DVE: only same-queue deps -> never blocks
                # the drain stream at the queue head.
                k0 = g * GW
                c_ap = ruT_tiles[i][0:OUT, k0:k0 + GW]
                s_ap = stT_tiles[i][:, k0:k0 + GW]
                d = cscr_pool.tile([OUT, GW], f16, tag="cscr")
                nc.vector.tensor_sub(d[:], c_ap, s_ap)
                t = cscr_pool.tile([OUT, GW], f16, tag="cscr")
                nc.vector.tensor_mul(t[:], um0_tiles[i][:, k0:k0 + GW], d[:])
                nc.vector.tensor_add(s_ap, s_ap, t[:])
                if g == NG - 1:
                    nc.sync.dma_start(outT_d[i], stT_tiles[i][:])

            big_phase(
                xc_t, wc_t, OUT, bc_ap, AF.Tanh, 1.0,
                lambda i, k0, w: ruT_tiles[i][0:OUT, k0:k0 + w],
                agg_rr=[v_drain, s_drain],
                aggT_rr=[nc.vector.tensor_copy, nc.scalar.copy],
                post_act=phase2_post_act,
                tp_bufs=3)

    nc.compile()
    return nc


def _get_module():
    if "nc" not in _CACHE:
        _CACHE["nc"] = _build_module()
    return _CACHE["nc"]


def pack_inputs(input, state, supports, Wr, br, Wu, bu, Wc, bc):
    """Host-side packing shared by kernel() and local sim harnesses."""
    input = np.asarray(input, np.float32)
    state = np.asarray(state, np.float32)
    supports = np.asarray(supports, np.float32)
    Wr = np.asarray(Wr, np.float32)
    br = np.asarray(br, np.float32)
    Wu = np.asarray(Wu, np.float32)
    bu = np.asarray(bu, np.float32)
    Wc = np.asarray(Wc, np.float32)
    bc = np.asarray(bc, np.float32)

    f8 = ml_dtypes.float8_e4m3

    # ST[j][m, k] = S[j][k, m] * SCALE; hi/lo split for the 3-product
    # compensated aggregation; swizzled [j, g, p, mt, slot(Sh16|Sh|Sl), k']
    stf = supports.transpose(0, 2, 1) * SCALE
    sh = stf.astype(f8)
    sl = (16.0 * (stf - sh.astype(np.float32))).astype(f8)
    sh16 = (sh.astype(np.float32) * 16.0).astype(f8)

    def _st_sw(a):
        # [J, m, k] -> [J, g, p, mt, k'] with m = mt*128 + p
        return a.reshape(J, 2 * MQ, P, NG, GW).transpose(0, 3, 2, 1, 4)

    st_host = np.ascontiguousarray(
        np.stack([_st_sw(sh16), _st_sw(sh), _st_sw(sl)], axis=4))

    # wcat: [66, J*128 | J*64] = [Wr|Wu] per j then Wc per j, all / SCALE
    wru = (np.concatenate([Wr, Wu], axis=2) / SCALE).transpose(1, 0, 2)
    wcc = (Wc / SCALE).transpose(1, 0, 2)
    wcat = np.ascontiguousarray(np.concatenate(
        [wru.reshape(F, J * 2 * OUT), wcc.reshape(F, J * OUT)],
        axis=1)).astype(np.float16)

    # bvec: col0 = [br | -bu], col1 = [+1|-1], col2 = [bc | 0]
    bvec = np.zeros((P, 3), np.float32)
    bvec[:, 0] = np.concatenate([br, -bu])
    bvec[:, 1] = np.concatenate([np.ones(OUT), -np.ones(OUT)])
    bvec[0:OUT, 2] = bc

    xs_full = np.concatenate([input, state], axis=2)  # [B, N, F]

    in_maps = []
    for c in range(NCORES):
        csl = slice(c * BC, (c + 1) * BC)
        # [N, Bc, F] -> [p, h, mt, slot(xl|xh), 264] with m = mt*128 + p
        xsc = xs_full[csl].transpose(1, 0, 2).reshape(2 * MQ, P, 2, HALF)
        xh = xsc.astype(f8)
        xl = (16.0 * (xsc - xh.astype(np.float32))).astype(f8)
        xs_c = np.ascontiguousarray(
            np.stack([xl, xh], axis=3).transpose(1, 2, 0, 3, 4))
        stT_c = np.ascontiguousarray(
            state[csl].transpose(0, 2, 1)).astype(np.float16)
        in_maps.append({
            "st": st_host,
            "xs": xs_c,
            "stT": stT_c,
            "wcat": wcat,
            "bvec": bvec,
        })
    return in_maps


def kernel(input, state, supports, Wr, br, Wu, bu, Wc, bc):
    from concourse.bass_utils import run_bass_kernel_spmd

    nc = _get_module()
    in_maps = pack_inputs(input, state, supports, Wr, br, Wu, bu, Wc, bc)

    import time
    t0 = time.monotonic()
    res = run_bass_kernel_spmd(nc, in_maps, core_ids=list(range(NCORES)))
    _CACHE["last_wall_s"] = time.monotonic() - t0

    out = np.empty((B, N, OUT), np.float32)
    for c in range(NCORES):
        outT = res.results[c]["outT"]           # [BC, OUT, N] f16
        out[c * BC:(c + 1) * BC] = outT.transpose(0, 2, 1).astype(np.float32)
    return out


# revision 47
# speedup vs baseline: 1.5308x; 1.0312x over previous
"""DCGRU cell Trainium2 kernel (fp8 DoubleRow aggregation).

Math (per batch i):
  xs = [input, state]                                  [N, 66]
  aggr[j] = S[j] @ xs          (J=4 supports)          [N, 66]
  r = sigmoid(sum_j aggr[j] @ Wr[j] + br)              [N, 64]
  um = 1 - sigmoid(sum_j aggr[j] @ Wu[j] + bu)         (= 1-u, via scale=-1)
  xc = [input, r*state]
  c = tanh(sum_j (S[j] @ xc) @ Wc[j] + bc)
  out = (1-um)*state + um*c = state + um*(c - state)

Sharding: data-parallel over batch, 8 batches per core on 8 cores.
supports/weights replicated. No collectives.

Device kernel (per core, Bc=8):
  - 3-product compensated fp8 aggregation (full-fp8 alone misses the
    2e-2 gate at ~8e-2): S ~ (Sh + Sl/16)/64, x ~ xh + xl/16, with
    psum = Sh16@xh (DoubleRow m-pairs, slot0) + (Sh@xl + Sl@xh) (one
    DoubleRow instruction per m-tile, slots), all e4m3. 24 instructions
    per [128, 264] psum half vs 16 for fp16 -- 25% less PE time at
    near-fp16 accuracy (max rel ~2.5e-3). Drains scale by 1/16; the
    remaining 1/64 is folded into the fp16 weights on host.
  - Supports host-swizzled [j, g, mh, p, mt, slot(Sh16|Sh|Sl), k] so
    every stationary half-tile DMA is fully contiguous per partition.
  - xs packed [p, h, mt, slot(xl|xh), 264] fp8, two DMAs.
  - Tail per group g (pipelined one group behind agg): per batch i issue
    act(i-2) -> transposes(i) + drains -> proj(i-1), so PE never waits on
    the cross-engine drain/act latency.
  - post_act hook spreads phase-boundary and combine work across groups:
    phase 1: (r*state).T chunk -> fp8 hi/lo split (DVE, same-queue
    chain) -> transposes -> xc fp8 slots;
    phase 2: out chunk = state + um*(c - state), DMA at last group.
  - Engine queue discipline throughout: all PSUM drains on DVE/Scalar
    (GPSIMD cannot access PSUM on hw); cross-engine dependency chains
    stay within one queue so an in-order queue head never blocks a
    latency-critical drain; issue order ~= execution order per queue
    (agg j-blocks interleaved with two tail iterations each).
"""

import sys

if '/opt/trn_rl_repo' not in sys.path:
    sys.path.insert(0, '/opt/trn_rl_repo')

import numpy as np
import ml_dtypes

B, N, IN, OUT, J = 64, 2048, 2, 64, 4
NCORES = 8
BC = B // NCORES            # 8 batches per core
F = IN + OUT                # 66
CB = BC * F                 # 528 moving columns
P = 128
HALF = CB // 2              # 264 (psum bank split)
NG = 8                      # k groups (256 cols each)
KBG = 2                     # k blocks per group
GW = KBG * P                # 256 cols per group
MQ = 8                      # m pair-tiles (each pair = 256 rows)
SCALE = 64.0                # fp8 support prescale (folded out via W/SCALE)

_CACHE = {}


def _build_module():
    import concourse.tile as tile
    import concourse.mybir as mybir
    from concourse import bacc
    from concourse.masks import make_identity

    f32 = mybir.dt.float32
    f16 = mybir.dt.float16
    f8 = mybir.dt.float8e4
    AF = mybir.ActivationFunctionType
    PM = mybir.MatmulPerfMode.DoubleRow

    nc = bacc.Bacc("TRN2", target_bir_lowering=False, debug=False,
                   num_devices=1)

    st_d = nc.dram_tensor("st", [J, NG, 2, P, MQ, 3, GW], f8,
                          kind="ExternalInput").ap()
    xs_d = nc.dram_tensor("xs", [P, 2, 2 * MQ, 2, HALF], f8,
                          kind="ExternalInput").ap()
    stT_d = nc.dram_tensor("stT", [BC, OUT, N], f16, kind="ExternalInput").ap()
    wcat_d = nc.dram_tensor("wcat", [F, J * 2 * OUT + J * OUT], f16,
                            kind="ExternalInput").ap()
    bvec_d = nc.dram_tensor("bvec", [P, 3], f32, kind="ExternalInput").ap()
    outT_d = nc.dram_tensor("outT", [BC, OUT, N], f16,
                            kind="ExternalOutput").ap()

    with tile.TileContext(nc) as tc:
        with tc.tile_pool(name="const", bufs=1) as const_pool, \
             tc.tile_pool(name="xs", bufs=1) as xs_pool, \
             tc.tile_pool(name="xc", bufs=1) as xc_pool, \
             tc.tile_pool(name="ruT", bufs=BC) as ruT_pool, \
             tc.tile_pool(name="stT", bufs=BC) as stT_pool, \
             tc.tile_pool(name="um0", bufs=BC) as um0_pool, \
             tc.tile_pool(name="rsT", bufs=2) as rsT_pool, \
             tc.tile_pool(name="cscr", bufs=2) as cscr_pool:

            # xs first (biggest serial dependency for the first matmuls);
            # wcat/bvec DMAs deferred into phase1_extra(0) behind the g0
            # support tiles.
            xs_t = xs_pool.tile([P, 2, 2 * MQ, 2, HALF], f8, tag="xs")
            nc.sync.dma_start(xs_t[:, 0], xs_d[:, 0])
            nc.sync.dma_start(xs_t[:, 1], xs_d[:, 1])

            def xs_load():
                pass
            wcat_t = const_pool.tile([F, J * 2 * OUT + J * OUT], f16,
                                     tag="wcat")
            bvec_t = const_pool.tile([P, 3], f32, tag="bvec")

            ident = const_pool.tile([P, P], f16, tag="ident")
            make_identity(nc, ident[:])

            wru_t = [wcat_t[:, j * 2 * OUT:(j + 1) * 2 * OUT]
                     for j in range(J)]
            wc_t = [wcat_t[:, J * 2 * OUT + j * OUT:J * 2 * OUT + (j + 1) * OUT]
                    for j in range(J)]
            bru_ap = bvec_t[:, 0:1]
            sgn_ap = bvec_t[:, 1:2]
            bc_ap = bvec_t[0:OUT, 2:3]

            xc_t = xc_pool.tile([P, 2, 2 * MQ, 2, HALF], f8, tag="xc")
            for h in range(2):
                dst = xc_t[:, h].rearrange("p m two (i f) -> p m two i f", f=F)
                src = xs_t[:, h].rearrange("p m two (i f) -> p m two i f", f=F)
                nc.gpsimd.tensor_copy(dst[:, :, :, :, 0:IN],
                                      src[:, :, :, :, 0:IN])

            stT_tiles = [stT_pool.tile([OUT, N], f16, tag="stT",
                                       name=f"stT{i}")
                         for i in range(BC)]
            ruT_tiles = [ruT_pool.tile([P, N], f16, tag="ruT", name=f"ruT{i}")
                         for i in range(BC)]
            um0_tiles = [um0_pool.tile([OUT, N], f16, tag="um0",
                                       name=f"um0{i}")
                         for i in range(BC)]

            def big_phase(x_t, w_tiles, out_rows, bias_ap, act_fn,
                          scale_ap, out_slice_fn, agg_rr, aggT_rr,
                          extra_fn=None, post_act=None, tp_bufs=4,
                          pre_fn=None):
                """One graph-conv pass + projection + activation.

                Pipelined: agg(g) is issued before tail(g-1); within a tail,
                act(i-2) -> post_act(i-2) -> transposes(i)+drains ->
                proj(i-1). Queue discipline: gp takes the slack-tolerant agg
                psum drains; scalar/DVE take the latency-critical tail work;
                post_act chains stay within one queue so they never block a
                drain at an in-order queue head.
                """
                with tc.tile_pool(name="stst", bufs=8) as st_pool, \
                     tc.tile_pool(name="agg", bufs=16) as agg_pool, \
                     tc.tile_pool(name="aggT", bufs=4) as aggT_pool, \
                     tc.tile_pool(name="aggps", bufs=2, space="PSUM") as agg_ps_pool, \
                     tc.tile_pool(name="tpps", bufs=tp_bufs, space="PSUM") as tp_ps_pool, \
                     tc.tile_pool(name="pjps", bufs=2, space="PSUM") as pj_ps_pool:

                    nd = 0
                    td = 0
                    pend_proj = [None]
                    pend_act = [None]

                    def agg_block(g, j, pre_fn=None):
                        # 3-product compensated fp8 aggregation for one
                        # support: psum = Sh16@xh (m-pairs, slot0) +
                        # (Sh@xl + Sl@xh) per m-tile (slots 1,2 x 0,1);
                        # drain scales by 1/16 -> aggr = 64*S@x in f16.
                        nonlocal nd
                        blk = {}
                        st_a = st_pool.tile([P, MQ, 3, GW], f8, tag="st")
                        nc.sync.dma_start(st_a[:], st_d[j, g, 0])
                        if pre_fn is not None:
                            pre_fn()
                        st_b = st_pool.tile([P, MQ, 3, GW], f8, tag="st")
                        nc.sync.dma_start(st_b[:], st_d[j, g, 1])
                        st_h = [st_a, st_b]
                        for kb in range(KBG):
                            t = agg_pool.tile([P, CB], f16, tag="agg",
                                              name=f"agg{j}_{kb}")
                            for h in range(2):
                                pst = agg_ps_pool.tile(
                                    [P, HALF], f32, tag="aggps",
                                    name=f"aggps{kb}_{h}")
                                for mq in range(MQ):
                                    mh, ml = divmod(mq, MQ // 2)
                                    nc.tensor.matmul(
                                        pst[:],
                                        st_h[mh][:, 2 * ml:2 * ml + 2, 0,
                                                 kb * P:(kb + 1) * P],
                                        x_t[:, h, 2 * mq:2 * mq + 2, 1, :],
                                        start=(mq == 0),
                                        stop=False,
                                        perf_mode=PM,
                                    )
                                for mt in range(2 * MQ):
                                    mh, ml = divmod(mt, MQ)
                                    nc.tensor.matmul(
                                        pst[:],
                                        st_h[mh][:, ml, 1:3,
                                                 kb * P:(kb + 1) * P],
                                        x_t[:, h, mt, :, :],
                                        start=False,
                                        stop=(mt == 2 * MQ - 1),
                                        perf_mode=PM,
                                    )
                                eng = agg_rr[nd % len(agg_rr)]
                                nd += 1
                                eng(t[:, h * HALF:(h + 1) * HALF], pst[:])
                            blk[(j, kb)] = t
                        return blk

                    def one_act(g):
                        pi, ppp = pend_act[0]
                        nc.scalar.activation(
                            out_slice_fn(pi, g * GW, GW), ppp[:],
                            act_fn, bias=bias_ap, scale=scale_ap)
                        pend_act[0] = None
                        if post_act is not None:
                            post_act(pi, g, tp_ps_pool)

                    def issue_proj(g):
                        pi, tpair = pend_proj[0]
                        pp = pj_ps_pool.tile([out_rows, GW], f32, tag="pj",
                                             name=f"proj{pi}")
                        for j in range(J):
                            nc.tensor.matmul(
                                pp[:], w_tiles[j][:, 0:out_rows],
                                tpair[j // 2][:, j % 2],
                                start=(j == 0), stop=(j == J - 1))
                        pend_proj[0] = None
                        pend_act[0] = (pi, pp)

                    def tail_iter(g, i, agg_sb):
                        nonlocal td
                        if pend_act[0] is not None:
                            one_act(g)      # act(i-2)
                        tpair = []
                        for half in range(2):
                            tp = tp_ps_pool.tile([F, 2, GW], f16,
                                                 tag="tppair",
                                                 name=f"tp{i}_{half}")
                            for jj in range(2):
                                j = half * 2 + jj
                                for kb in range(KBG):
                                    nc.tensor.transpose(
                                        tp[:, jj, kb * P:(kb + 1) * P],
                                        agg_sb[(j, kb)][:, i * F:(i + 1) * F],
                                        ident[:])
                            at = aggT_pool.tile([F, 2, GW], f16, tag="aggT",
                                                name=f"aggT{i}_{half}")
                            aggT_rr[td % len(aggT_rr)](at[:], tp[:])
                            td += 1
                            tpair.append(at)
                        if pend_proj[0] is not None:
                            issue_proj(g)   # proj(i-1)
                        pend_proj[0] = (i, tpair)

                    def tail_flush(g):
                        # act(BC-2), proj(BC-1), act(BC-1)
                        if pend_act[0] is not None:
                            one_act(g)
                        issue_proj(g)
                        one_act(g)

                    # Interleaved issue: each engine queue sees work in
                    # (approximate) temporal execution order, so no queue
                    # head ever waits on far-future work while near-term
                    # work sits behind it.
                    agg_prev = None
                    for g in range(NG):
                        agg_cur = {}
                        for j in range(J):
                            agg_cur.update(agg_block(
                                g, j,
                                pre_fn if (g == 0 and j == 0) else None))
                            if j == 0 and extra_fn is not None:
                                extra_fn(g)
                            if agg_prev is not None:
                                tail_iter(g - 1, 2 * j, agg_prev)
                                tail_iter(g - 1, 2 * j + 1, agg_prev)
                        if agg_prev is not None:
                            tail_flush(g - 1)
                        agg_prev = agg_cur
                    for i in range(BC):
                        tail_iter(NG - 1, i, agg_prev)
                    tail_flush(NG - 1)

            # ---------------- phase 1 ----------------
            def phase1_extra(g):
                if g == 0:
                    nc.sync.dma_start(wcat_t[:], wcat_d[:])
                    nc.sync.dma_start(bvec_t[:], bvec_d[:])
                if g <= 1:
                    for i in range(g * 4, g * 4 + 4):
                        nc.sync.dma_start(stT_tiles[i][:], stT_d[i])

            rst_n = [0]

            def phase1_post_act(i, g, tp_pool):
                # (r*state).T chunk, split hi/lo, transposed into xc fp8:
                #   rsh8 = fp8(rst); res = rst - f16(rsh8); xl = fp8(16*res)
                # hi and res transposed separately; drains convert to fp8.
                k0 = g * GW
                h, il = divmod(i, 4)
                rst = rsT_pool.tile([OUT, GW], f16, tag="rsT")
                nc.vector.tensor_mul(rst[:], ruT_tiles[i][0:OUT, k0:k0 + GW],
                                     stT_tiles[i][:, k0:k0 + GW])
                rsh8 = rsT_pool.tile([OUT, GW], f8, tag="rs8")
                nc.vector.tensor_copy(rsh8[:], rst[:])
                rsh16 = rsT_pool.tile([OUT, GW], f16, tag="rsH")
                nc.vector.tensor_copy(rsh16[:], rsh8[:])
                res = rsT_pool.tile([OUT, GW], f16, tag="rsR")
                nc.vector.tensor_sub(res[:], rst[:], rsh16[:])
                tph = tp_pool.tile([P, KBG, OUT], f16, tag="tppair",
                                   name=f"rsth{i}")
                tpl = tp_pool.tile([P, KBG, OUT], f16, tag="tppair",
                                   name=f"rstl{i}")
                for kb in range(KBG):
                    nc.tensor.transpose(tph[:, kb],
                                        rsh16[:, kb * P:(kb + 1) * P],
                                        ident[0:OUT, 0:OUT])
                    nc.tensor.transpose(tpl[:, kb],
                                        res[:, kb * P:(kb + 1) * P],
                                        ident[0:OUT, 0:OUT])
                for kb in range(KBG):
                    mt = g * KBG + kb
                    ds = xc_t[:, h, mt, :, il * F + IN:(il + 1) * F]
                    if rst_n[0] % 2 == 0:
                        nc.vector.tensor_copy(ds[:, 1], tph[:, kb])
                        nc.scalar.activation(ds[:, 0], tpl[:, kb], AF.Copy,
                                             scale=16.0)
                    else:
                        nc.scalar.copy(ds[:, 1], tph[:, kb])
                        nc.vector.tensor_scalar_mul(ds[:, 0], tpl[:, kb],
                                                    16.0)
                    rst_n[0] += 1

            def v_drain(dst, src_ap):
                nc.vector.tensor_scalar_mul(dst, src_ap, 1.0 / 16.0)

            def s_drain(dst, src_ap):
                nc.scalar.activation(dst, src_ap, AF.Copy, scale=1.0 / 16.0)

            big_phase(
                xs_t, wru_t, 2 * OUT, bru_ap, AF.Sigmoid, sgn_ap,
                lambda i, k0, w: ruT_tiles[i][:, k0:k0 + w],
                agg_rr=[v_drain, s_drain],
                aggT_rr=[nc.vector.tensor_copy, nc.scalar.copy],
                extra_fn=phase1_extra,
                post_act=phase1_post_act,
                tp_bufs=4,
                pre_fn=xs_load)

            # um = (1-u).T needs partition base 0: SBUF->SBUF DMA via the
            # Activation queue (SP is busy prefetching phase-2 supports).
            for i in range(BC):
                nc.gpsimd.dma_start(um0_tiles[i][:],
                                    ruT_tiles[i][OUT:2 * OUT, :])

            # ---------------- phase 2 ----------------
            def phase2_post_act(i, g, pj_pool):
                # out chunk = state + um*(c - state), in place over stT.
                # Whole chain on DVE: only same-queue deps -> never blocks
                # the drain stream at the queue head.
                k0 = g * GW
                c_ap = ruT_tiles[i][0:OUT, k0:k0 + GW]
                s_ap = stT_tiles[i][:, k0:k0 + GW]
                d = cscr_pool.tile([OUT, GW], f16, tag="cscr")
                nc.vector.tensor_sub(d[:], c_ap, s_ap)
                t = cscr_pool.tile([OUT, GW], f16, tag="cscr")
                nc.vector.tensor_mul(t[:], um0_tiles[i][:, k0:k0 + GW], d[:])
                nc.vector.tensor_add(s_ap, s_ap, t[:])
                if g == NG - 2:
                    # store all but the last chunk early; shrinks the tail
                    nc.sync.dma_start(outT_d[i][:, 0:(NG - 1) * GW],
                                      stT_tiles[i][:, 0:(NG - 1) * GW])
                elif g == NG - 1:
                    nc.sync.dma_start(outT_d[i][:, (NG - 1) * GW:],
                                      stT_tiles[i][:, (NG - 1) * GW:])

            big_phase(
                xc_t, wc_t, OUT, bc_ap, AF.Tanh, 1.0,
                lambda i, k0, w: ruT_tiles[i][0:OUT, k0:k0 + w],
                agg_rr=[v_drain, s_drain],
                aggT_rr=[nc.vector.tensor_copy, nc.scalar.copy],
                post_act=phase2_post_act,
                tp_bufs=4)

    nc.compile()
    return nc


def _get_module():
    if "nc" not in _CACHE:
        _CACHE["nc"] = _build_module()
    return _CACHE["nc"]


def pack_inputs(input, state, supports, Wr, br, Wu, bu, Wc, bc):
    """Host-side packing shared by kernel() and local sim harnesses."""
    input = np.asarray(input, np.float32)
    state = np.asarray(state, np.float32)
    supports = np.asarray(supports, np.float32)
    Wr = np.asarray(Wr, np.float32)
    br = np.asarray(br, np.float32)
    Wu = np.asarray(Wu, np.float32)
    bu = np.asarray(bu, np.float32)
    Wc = np.asarray(Wc, np.float32)
    bc = np.asarray(bc, np.float32)

    f8 = ml_dtypes.float8_e4m3

    # ST[j][m, k] = S[j][k, m] * SCALE; hi/lo split for the 3-product
    # compensated aggregation; swizzled [j, g, p, mt, slot(Sh16|Sh|Sl), k']
    stf = supports.transpose(0, 2, 1) * SCALE
    sh = stf.astype(f8)
    sl = (16.0 * (stf - sh.astype(np.float32))).astype(f8)
    sh16 = (sh.astype(np.float32) * 16.0).astype(f8)

    def _st_sw(a):
        # [J, m, k] -> [J, g, p, mt, k'] with m = mt*128 + p
        return a.reshape(J, 2 * MQ, P, NG, GW).transpose(0, 3, 2, 1, 4)

    st_host = np.stack([_st_sw(sh16), _st_sw(sh), _st_sw(sl)], axis=4)
    st_host = np.ascontiguousarray(
        st_host.reshape(J, NG, P, 2, MQ, 3, GW).transpose(
            0, 1, 3, 2, 4, 5, 6))

    # wcat: [66, J*128 | J*64] = [Wr|Wu] per j then Wc per j, all / SCALE
    wru = (np.concatenate([Wr, Wu], axis=2) / SCALE).transpose(1, 0, 2)
    wcc = (Wc / SCALE).transpose(1, 0, 2)
    wcat = np.ascontiguousarray(np.concatenate(
        [wru.reshape(F, J * 2 * OUT), wcc.reshape(F, J * OUT)],
        axis=1)).astype(np.float16)

    # bvec: col0 = [br | -bu], col1 = [+1|-1], col2 = [bc | 0]
    bvec = np.zeros((P, 3), np.float32)
    bvec[:, 0] = np.concatenate([br, -bu])
    bvec[:, 1] = np.concatenate([np.ones(OUT), -np.ones(OUT)])
    bvec[0:OUT, 2] = bc

    xs_full = np.concatenate([input, state], axis=2)  # [B, N, F]

    in_maps = []
    for c in range(NCORES):
        csl = slice(c * BC, (c + 1) * BC)
        # [N, Bc, F] -> [p, h, mt, slot(xl|xh), 264] with m = mt*128 + p
        xsc = xs_full[csl].transpose(1, 0, 2).reshape(2 * MQ, P, 2, HALF)
        xh = xsc.astype(f8)
        xl = (16.0 * (xsc - xh.astype(np.float32))).astype(f8)
        xs_c = np.ascontiguousarray(
            np.stack([xl, xh], axis=3).transpose(1, 2, 0, 3, 4))
        stT_c = np.ascontiguousarray(
            state[csl].transpose(0, 2, 1)).astype(np.float16)
        in_maps.append({
            "st": st_host,
            "xs": xs_c,
            "stT": stT_c,
            "wcat": wcat,
            "bvec": bvec,
        })
    return in_maps


def kernel(input, state, supports, Wr, br, Wu, bu, Wc, bc):
    from concourse.bass_utils import run_bass_kernel_spmd

    nc = _get_module()
    in_maps = pack_inputs(input, state, supports, Wr, br, Wu, bu, Wc, bc)

    import time
    t0 = time.monotonic()
    res = run_bass_kernel_spmd(nc, in_maps, core_ids=list(range(NCORES)))
    _CACHE["last_wall_s"] = time.monotonic() - t0

    out = np.empty((B, N, OUT), np.float32)
    for c in range(NCORES):
        outT = res.results[c]["outT"]           # [BC, OUT, N] f16
        out[c * BC:(c + 1) * BC] = outT.transpose(0, 2, 1).astype(np.float32)
    return out
